# revision 11
# baseline (speedup 1.0000x reference)
"""Single-head causal self-attention (B=8, T=2048, D=512, H=64), data-parallel
over batch across 8 NeuronCores (batch element = core id). 33.3us simulated
(2.12x over the 70.6us baseline), rel err 4.4e-3 on hardware.

Per-core design (ACT/exp-bound; bf16 operands, f32 PSUM accumulate):
  - x uploaded bf16; xT (d on partitions) loaded via hardware DMA transpose,
    one call per 512-wide t-chunk, alternating SP/ACT HWDGE queues so the
    global DMA completion-sem chain overlaps (arrivals: tc0, tc2, tc1, tc3)
  - weights + biases packed in ONE bf16 DRAM tensor; bk dropped entirely
    (row-constant softmax terms cancel), bq*scale / bv folded into the
    projection PSUM->SBUF copies; tc0's k-copy runs on the then-idle ACT
    engine (it gates the first exp), the rest on DVE
  - packed kq projection [128, T] (k parts 0:64, q 64:128); v as vT [64, T]
    then PE-transposed into v_aug [128, 16, 65] whose ones column accumulates
    the softmax denominator during PV
  - S^T per 512-wide i-block with exact causal trim: full j-tile pairs in
    [128,1024] PSUM slots; diagonal pieces split A=[m0: 512] /
    B=[m1: 384, m3: 128, m2: 256] so both diag exps are long enough to hide
    the 2-buffer PSUM slot turnaround; exp on ACT writes bf16 SBUF; the
    128x128 diagonal bands are masked by a DVE multiply with a tri matrix
  - one exp tile (block-3 pair 0) computes exp as gpsimd pow(e, x) on the
    otherwise-idle Pool engine (staged to SBUF by DVE, which also frees the
    PSUM slot), shortening the ACT stream
  - PV flipped per i-tile (O[128,65] accumulates with e2 slices stationary,
    diag pieces first): output in row layout, no output transposes; epilogue
    is a batched reciprocal + broadcast multiply, bf16 out DMA per block
  - dummy-matmul warmup keeps PE busy through the DMA lead-in so the p-state
    ramp (which resets on idle) completes before real matmuls arrive
  - instruction order forced via tile_wait_until slots (arrival-aware
    software pipeline); the exp stream runs near-gapless from ~8.8us to ~28us
"""

import sys

for _p in ("/root/.axon_site/_ro/trn_rl_repo", "/opt/trn_rl_repo"):
    if _p not in sys.path:
        sys.path.append(_p)

import numpy as np
import ml_dtypes
import concourse.bass as bass
import concourse.bacc as bacc
import concourse.tile as tile
from concourse import mybir
from concourse.bass_utils import run_bass_kernel_spmd

F32 = mybir.dt.float32
BF16 = mybir.dt.bfloat16
EXP = mybir.ActivationFunctionType.Exp

B, T, D, H = 8, 2048, 512, 64
ND = D // 128
NT = T // 128
NB = T // 512

# diag piece placement: (slot, offset, n) for m = 0..3
# slot A: m0 [0:512] (ap 512); slot B: m1 [0:384], m3 [384:512], m2 [512:768]
# (ap 768) — both exps long enough to hide the psS slot turnaround
DIAG = [(0, 0, 512), (1, 0, 384), (1, 512, 256), (1, 384, 128)]


def build_body(nc, tc, ctx, dram, repeat=1):
    x_d, w_d, out_d = dram

    persist = ctx.enter_context(tc.tile_pool(name="persist", bufs=1))
    epool = ctx.enter_context(tc.tile_pool(name="epool", bufs=12))
    spool = ctx.enter_context(tc.tile_pool(name="spool", bufs=3))
    rpool = ctx.enter_context(tc.tile_pool(name="rpool", bufs=2))
    psK = ctx.enter_context(tc.tile_pool(name="psK", bufs=2, space="PSUM"))
    psS = ctx.enter_context(tc.tile_pool(name="psS", bufs=2, space="PSUM"))
    psX = ctx.enter_context(tc.tile_pool(name="psX", bufs=1, space="PSUM"))
    psO = ctx.enter_context(tc.tile_pool(name="psO", bufs=1, space="PSUM"))

    # --- PE warmup through the DMA lead-in ---
    wu = persist.tile([1, 512], BF16)
    nc.vector.memset(wu[:], 0.0)
    for i in range(10):
        wu_ps = psK.tile([16, 512], F32, tag="k", name=f"wu{i}")
        nc.tensor.matmul(wu_ps[:], wu[:, 0:16], wu[:], start=True, stop=True)

    # --- persistent activations ---
    xT = [persist.tile([128, ND, 512], BF16, name=f"xT{t}") for t in range(4)]
    k_sb = persist.tile([H, T], BF16)
    q_sb = persist.tile([H, T], BF16)
    vT = persist.tile([H, T], BF16)
    v_aug = persist.tile([128, NT, H + 1], BF16)
    o_sb = persist.tile([128, NT, H], BF16)

    # --- constants ---
    w_all = persist.tile([128, ND * 192 + 2], BF16)
    b_all = persist.tile([H, 2], F32)

    tri = persist.tile([128, 128], BF16)
    nc.vector.memset(tri[:], 1.0)
    nc.gpsimd.affine_select(out=tri[:], in_=tri[:],
                            compare_op=mybir.AluOpType.is_ge, fill=0.0,
                            base=0, pattern=[[1, 128]], channel_multiplier=-1)

    ident = persist.tile([H, H], BF16)
    nc.vector.memset(ident[:], 1.0)
    nc.gpsimd.affine_select(out=ident[:], in_=ident[:],
                            compare_op=mybir.AluOpType.is_equal, fill=0.0,
                            base=0, pattern=[[1, H]], channel_multiplier=-1)

    nc.vector.memset(v_aug[:, :, H:H + 1], 1.0)

    ce = persist.tile([128, 1], F32)
    nc.vector.memset(ce[:], 2.718281828459045)

    for _rep in range(repeat):
        if _rep == 0:
            nc.gpsimd.dma_start(w_all[:], w_d[:])
            nc.vector.tensor_copy(b_all[:], w_all[0:H, ND * 192:ND * 192 + 2])
        for tch in range(4):
            tsl = slice(tch * 512, (tch + 1) * 512)
            eng = nc.sync if tch % 2 == 0 else nc.scalar
            eng.dma_start_transpose(xT[tch][:], x_d[tsl, :])

        kq_ps_all = {}
        e_full = {}   # (b, g) -> tile; g = jt//2
        e_diag = {}   # (b, sl) -> tile
        o_ps_all = {}

        def emit_proj_kq(tch):
            tsl = slice(tch * 512, (tch + 1) * 512)
            kq_ps = psK.tile([128, 512], F32, tag="k", name=f"kq{tch}")
            for dc in range(ND):
                nc.tensor.matmul(kq_ps[:], w_all[:, dc * 192:dc * 192 + 128],
                                 xT[tch][:, dc, :],
                                 start=(dc == 0), stop=(dc == ND - 1))
            nc.vector.tensor_scalar_add(q_sb[:, tsl], kq_ps[H:128, :],
                                        b_all[:, 0:1])
            kq_ps_all[tch] = kq_ps

        def emit_kcopy(tch, on_act):
            tsl = slice(tch * 512, (tch + 1) * 512)
            kq_ps = kq_ps_all[tch]
            if on_act:
                nc.scalar.copy(k_sb[:, tsl], kq_ps[0:H, :])
            else:
                nc.vector.tensor_copy(k_sb[:, tsl], kq_ps[0:H, :])

        def emit_proj_v(tch):
            tsl = slice(tch * 512, (tch + 1) * 512)
            v_ps = psX.tile([H, 512], F32, tag="px")
            for dc in range(ND):
                nc.tensor.matmul(v_ps[:],
                                 w_all[:, dc * 192 + 128:dc * 192 + 192],
                                 xT[tch][:, dc, :],
                                 start=(dc == 0), stop=(dc == ND - 1))
            nc.vector.tensor_scalar_add(vT[:, tsl], v_ps[:], b_all[:, 1:2])
            vt_ps = psX.tile([128, 4, H], BF16, tag="px")
            for q in range(4):
                jt = 4 * tch + q
                nc.tensor.transpose(vt_ps[:, q, :],
                                    vT[:, jt * 128:(jt + 1) * 128], ident[:])
            nc.vector.tensor_copy(v_aug[:, 4 * tch:4 * tch + 4, 0:H], vt_ps[:])

        def emit_spair(b, g, on_pool=False):
            isl = slice(b * 512, (b + 1) * 512)
            st = psS.tile([128, 1024], F32, tag="st", name=f"stf{b}_{g}")
            for h in range(2):
                jt = 2 * g + h
                nc.tensor.matmul(st[:, 512 * h:512 * (h + 1)],
                                 k_sb[:, jt * 128:(jt + 1) * 128],
                                 q_sb[:, isl], start=True, stop=True)
            ef = epool.tile([128, 1024], BF16, tag="e", name=f"ef{b}_{g}")
            if on_pool:
                s2 = spool.tile([128, 1024], F32, tag="s2", name=f"s2_{b}_{g}")
                nc.vector.tensor_copy(s2[:], st[:])
                nc.gpsimd.tensor_tensor(ef[:], ce[:].broadcast_to((128, 1024)),
                                        s2[:], mybir.AluOpType.pow)
            else:
                nc.scalar.activation(ef[:], st[:], EXP)
            e_full[(b, g)] = ef

        def emit_sdiag(b, sl):
            ms = (0,) if sl == 0 else (1, 3, 2)
            st = psS.tile([128, 1024], F32, tag="st", name=f"std{b}_{sl}")
            used = 512 if sl == 0 else 768
            for m in ms:
                _, off, n = DIAG[m]
                jt = 4 * b + m
                nc.tensor.matmul(
                    st[:, off:off + n],
                    k_sb[:, jt * 128:(jt + 1) * 128],
                    q_sb[:, b * 512 + 128 * m:(b + 1) * 512],
                    start=True, stop=True)
            ed = epool.tile([128, 1024], BF16, tag="e", name=f"ed{b}_{sl}")
            nc.scalar.activation(ed[:, 0:used], st[:, 0:used], EXP)
            for m in ms:
                _, off, _ = DIAG[m]
                nc.vector.tensor_tensor(ed[:, off:off + 128],
                                        ed[:, off:off + 128], tri[:],
                                        mybir.AluOpType.mult)
            e_diag[(b, sl)] = ed

        def emit_pv(b, m_loc):
            if m_loc == 0:
                o_ps_all[b] = psO.tile([128, 4, H + 1], F32, tag="o",
                                       name=f"o_ps{b}")
            o_ps = o_ps_all[b]
            it = 4 * b + m_loc
            order = [4 * b + m for m in range(m_loc + 1)] + \
                    [jt for jt in range(4 * b)]
            for idx, jt in enumerate(order):
                if jt < 4 * b:
                    src = e_full[(b, jt // 2)]
                    col = 512 * (jt % 2) + 128 * m_loc
                else:
                    m = jt - 4 * b
                    sl, off, _ = DIAG[m]
                    src = e_diag[(b, sl)]
                    col = off + 128 * (m_loc - m)
                nc.tensor.matmul(o_ps[:, m_loc, :],
                                 src[:, col:col + 128],
                                 v_aug[:, jt, :],
                                 start=(idx == 0), stop=(idx == len(order) - 1))

        def emit_epilogue(b):
            o_ps = o_ps_all[b]
            rec = rpool.tile([128, 4], F32, tag="r")
            nc.vector.reciprocal(rec[:], o_ps[:, :, H:H + 1].rearrange(
                "p a o -> p (a o)"))
            rec_b = rec[:].unsqueeze(2).broadcast_to((128, 4, H))
            nc.vector.tensor_tensor(o_sb[:, 4 * b:4 * b + 4, :],
                                    o_ps[:, :, 0:H], rec_b,
                                    mybir.AluOpType.mult)
            isl = slice(b * 512, (b + 1) * 512)
            nc.sync.dma_start(
                out_d[isl, :].rearrange("(a p) h -> p a h", p=128),
                o_sb[:, 4 * b:4 * b + 4, :])

        # arrival-aware emission, order forced via tile_wait_until slots
        SCHED = [
            (4.7, lambda: emit_proj_kq(0)),
            (6.2, lambda: emit_kcopy(0, True)),
            (6.8, lambda: emit_sdiag(0, 0)),
            (7.0, lambda: emit_sdiag(0, 1)),
            (7.1, lambda: emit_proj_kq(2)),
            (7.9, lambda: emit_kcopy(2, False)),
            (7.8, lambda: emit_proj_v(0)),
            (9.4, lambda: emit_proj_kq(1)),
            (9.6, lambda: emit_kcopy(1, False)),
            (9.7, lambda: emit_sdiag(2, 0)),
            (9.9, lambda: emit_sdiag(2, 1)),
            (10.0, lambda: emit_spair(2, 0)),
            (10.2, lambda: emit_spair(2, 1)),
            (10.3, lambda: emit_pv(0, 0)),
            (10.35, lambda: emit_pv(0, 1)),
            (10.4, lambda: emit_pv(0, 2)),
            (10.45, lambda: emit_pv(0, 3)),
            (10.5, lambda: emit_proj_v(2)),
            (10.8, lambda: emit_sdiag(1, 0)),
            (11.0, lambda: emit_sdiag(1, 1)),
            (11.1, lambda: emit_spair(1, 0)),
            (11.3, lambda: emit_spair(1, 1)),
            (11.35, lambda: emit_epilogue(0)),
            (11.4, lambda: emit_proj_kq(3)),
            (11.55, lambda: emit_kcopy(3, False)),
            (11.6, lambda: emit_spair(2, 2)),
            (11.8, lambda: emit_spair(2, 3)),
            (11.9, lambda: emit_proj_v(1)),
            (12.3, lambda: emit_sdiag(3, 0)),
            (12.5, lambda: emit_sdiag(3, 1)),
            (12.6, lambda: emit_pv(1, 0)),
            (12.65, lambda: emit_pv(1, 1)),
            (12.7, lambda: emit_pv(1, 2)),
            (12.75, lambda: emit_pv(1, 3)),
            (12.8, lambda: emit_proj_v(3)),
            (12.9, lambda: emit_spair(3, 0, on_pool=True)),
            (13.1, lambda: emit_spair(3, 1)),
            (13.15, lambda: emit_epilogue(1)),
            (13.3, lambda: emit_pv(2, 0)),
            (13.35, lambda: emit_pv(2, 1)),
            (13.4, lambda: emit_pv(2, 2)),
            (13.45, lambda: emit_pv(2, 3)),
            (13.6, lambda: emit_spair(3, 2)),
            (13.8, lambda: emit_spair(3, 3)),
            (14.0, lambda: emit_spair(3, 4)),
            (14.2, lambda: emit_spair(3, 5)),
            (14.3, lambda: emit_epilogue(2)),
            (14.5, lambda: emit_pv(3, 0)),
            (14.55, lambda: emit_pv(3, 1)),
            (14.6, lambda: emit_pv(3, 2)),
            (14.65, lambda: emit_pv(3, 3)),
            (14.8, lambda: emit_epilogue(3)),
        ]
        for ts_us, fn in SCHED:
            with tc.tile_wait_until(ts_us * 1e-3):
                fn()

def build_nc(repeat=1):
    nc = bacc.Bacc("TRN2", target_bir_lowering=False, debug=False, num_devices=8)
    x_d = nc.dram_tensor("x", [T, D], BF16, kind="ExternalInput")
    w_d = nc.dram_tensor("w", [128, ND * 192 + 2], BF16, kind="ExternalInput")
    out_d = nc.dram_tensor("out", [T, H], BF16, kind="ExternalOutput")
    dram = (x_d, w_d, out_d)

    from contextlib import ExitStack
    with tile.TileContext(nc) as tc:
        with ExitStack() as ctx:
            build_body(nc, tc, ctx, dram, repeat=repeat)
    nc.compile()
    return nc


_NC_CACHE = {}


def _get_nc(repeat=1):
    if repeat not in _NC_CACHE:
        _NC_CACHE[repeat] = build_nc(repeat)
    return _NC_CACHE[repeat]


def make_in_maps(x, Wk, bk, Wq, bq, Wv, bv):
    scale = float(H) ** -0.5
    bf = ml_dtypes.bfloat16
    w = np.concatenate(
        [Wk.reshape(ND, 128, H), (Wq * scale).reshape(ND, 128, H),
         Wv.reshape(ND, 128, H)], axis=2)
    w = np.ascontiguousarray(w.transpose(1, 0, 2)).reshape(128, ND * 192)
    b = np.zeros((128, 2), dtype=np.float32)
    b[0:H, 0] = bq * scale
    b[0:H, 1] = bv
    w = np.concatenate([w, b], axis=1).astype(bf)
    xb = x.astype(bf)
    return [
        {"x": np.ascontiguousarray(xb[i]), "w": np.ascontiguousarray(w)}
        for i in range(B)
    ]


def kernel(x, Wk, bk, Wq, bq, Wv, bv, _repeat=1):
    x = np.asarray(x, dtype=np.float32)
    Wk = np.asarray(Wk, dtype=np.float32)
    bk = np.asarray(bk, dtype=np.float32)
    Wq = np.asarray(Wq, dtype=np.float32)
    bq = np.asarray(bq, dtype=np.float32)
    Wv = np.asarray(Wv, dtype=np.float32)
    bv = np.asarray(bv, dtype=np.float32)

    nc = _get_nc(_repeat)
    in_maps = make_in_maps(x, Wk, bk, Wq, bq, Wv, bv)
    res = run_bass_kernel_spmd(nc, in_maps, core_ids=list(range(B)))
    out = np.stack([np.asarray(res.results[i]["out"], dtype=np.float32)
                    for i in range(B)], axis=0)
    return out


# revision 12
# speedup vs baseline: 1.0023x; 1.0023x over previous
"""Single-head causal self-attention (B=8, T=2048, D=512, H=64), data-parallel
over batch across 8 NeuronCores — v9.

vs v8:
  - S^T slots are [128, 1024] pairs in their own pool (psS bufs=2) so the
    slot rotation alternates purely between S/exp: S(k) overlaps exp(k-1)
  - kq projection has its own psK pool (bufs=2): proj pipelines with DMA
  - diag slot A packs m0 [0:512], m1 [512:896], m3 [896:1024] (ap 1024,
    exact trim); slot B holds m2 [0:256]
  - PSUM: psK 2 + psS 4 + psX 1 + psO 1 = 8 banks
"""

import sys

for _p in ("/root/.axon_site/_ro/trn_rl_repo", "/opt/trn_rl_repo"):
    if _p not in sys.path:
        sys.path.append(_p)

import numpy as np
import ml_dtypes
import concourse.bass as bass
import concourse.bacc as bacc
import concourse.tile as tile
from concourse import mybir
from concourse.bass_utils import run_bass_kernel_spmd

F32 = mybir.dt.float32
BF16 = mybir.dt.bfloat16
EXP = mybir.ActivationFunctionType.Exp

B, T, D, H = 8, 2048, 512, 64
ND = D // 128
NT = T // 128
NB = T // 512

# diag piece placement: (slot, offset, n) for m = 0..3
# slot A: m0 [0:512] (ap 512); slot B: m1 [0:384], m3 [384:512], m2 [512:768]
# (ap 768) — both exps long enough to hide the psS slot turnaround
DIAG = [(0, 0, 512), (1, 0, 384), (1, 512, 256), (1, 384, 128)]


def build_body(nc, tc, ctx, dram, repeat=1):
    x_d, w_d, out_d = dram

    persist = ctx.enter_context(tc.tile_pool(name="persist", bufs=1))
    epool = ctx.enter_context(tc.tile_pool(name="epool", bufs=12))
    spool = ctx.enter_context(tc.tile_pool(name="spool", bufs=3))
    rpool = ctx.enter_context(tc.tile_pool(name="rpool", bufs=2))
    psK = ctx.enter_context(tc.tile_pool(name="psK", bufs=2, space="PSUM"))
    psS = ctx.enter_context(tc.tile_pool(name="psS", bufs=2, space="PSUM"))
    psX = ctx.enter_context(tc.tile_pool(name="psX", bufs=1, space="PSUM"))
    psO = ctx.enter_context(tc.tile_pool(name="psO", bufs=1, space="PSUM"))

    # --- PE warmup through the DMA lead-in ---
    wu = persist.tile([1, 512], BF16)
    nc.vector.memset(wu[:], 0.0)
    for i in range(10):
        wu_ps = psK.tile([16, 512], F32, tag="k", name=f"wu{i}")
        nc.tensor.matmul(wu_ps[:], wu[:, 0:16], wu[:], start=True, stop=True)

    # --- persistent activations ---
    xT = [persist.tile([128, ND, 512], BF16, name=f"xT{t}") for t in range(4)]
    k_sb = persist.tile([H, T], BF16)
    q_sb = persist.tile([H, T], BF16)
    vT = persist.tile([H, T], BF16)
    v_aug = persist.tile([128, NT, H + 1], BF16)
    o_sb = persist.tile([128, NT, H], BF16)

    # --- constants ---
    w_all = persist.tile([128, ND * 192 + 2], BF16)
    b_all = persist.tile([H, 2], F32)

    tri = persist.tile([128, 128], BF16)
    nc.vector.memset(tri[:], 1.0)
    nc.gpsimd.affine_select(out=tri[:], in_=tri[:],
                            compare_op=mybir.AluOpType.is_ge, fill=0.0,
                            base=0, pattern=[[1, 128]], channel_multiplier=-1)

    ident = persist.tile([H, H], BF16)
    nc.vector.memset(ident[:], 1.0)
    nc.gpsimd.affine_select(out=ident[:], in_=ident[:],
                            compare_op=mybir.AluOpType.is_equal, fill=0.0,
                            base=0, pattern=[[1, H]], channel_multiplier=-1)

    nc.vector.memset(v_aug[:, :, H:H + 1], 1.0)

    ce = persist.tile([128, 1], F32)
    nc.vector.memset(ce[:], 2.718281828459045)

    for _rep in range(repeat):
        if _rep == 0:
            nc.gpsimd.dma_start(w_all[:], w_d[:])
            nc.vector.tensor_copy(b_all[:], w_all[0:H, ND * 192:ND * 192 + 2])
        for tch in range(4):
            tsl = slice(tch * 512, (tch + 1) * 512)
            eng = nc.sync if tch % 2 == 0 else nc.scalar
            eng.dma_start_transpose(xT[tch][:], x_d[tsl, :])

        kq_ps_all = {}
        e_full = {}   # (b, g) -> tile; g = jt//2
        e_diag = {}   # (b, sl) -> tile
        o_ps_all = {}

        def emit_proj_kq(tch):
            tsl = slice(tch * 512, (tch + 1) * 512)
            kq_ps = psK.tile([128, 512], F32, tag="k", name=f"kq{tch}")
            for dc in range(ND):
                nc.tensor.matmul(kq_ps[:], w_all[:, dc * 192:dc * 192 + 128],
                                 xT[tch][:, dc, :],
                                 start=(dc == 0), stop=(dc == ND - 1))
            if tch == 0:
                nc.scalar.add(q_sb[:, tsl], kq_ps[H:128, :], b_all[:, 0:1])
            else:
                nc.vector.tensor_scalar_add(q_sb[:, tsl], kq_ps[H:128, :],
                                            b_all[:, 0:1])
            kq_ps_all[tch] = kq_ps

        def emit_kcopy(tch, on_act):
            tsl = slice(tch * 512, (tch + 1) * 512)
            kq_ps = kq_ps_all[tch]
            if on_act:
                nc.scalar.copy(k_sb[:, tsl], kq_ps[0:H, :])
            else:
                nc.vector.tensor_copy(k_sb[:, tsl], kq_ps[0:H, :])

        def emit_proj_v(tch):
            tsl = slice(tch * 512, (tch + 1) * 512)
            v_ps = psX.tile([H, 512], F32, tag="px")
            for dc in range(ND):
                nc.tensor.matmul(v_ps[:],
                                 w_all[:, dc * 192 + 128:dc * 192 + 192],
                                 xT[tch][:, dc, :],
                                 start=(dc == 0), stop=(dc == ND - 1))
            nc.vector.tensor_scalar_add(vT[:, tsl], v_ps[:], b_all[:, 1:2])
            vt_ps = psX.tile([128, 4, H], BF16, tag="px")
            for q in range(4):
                jt = 4 * tch + q
                nc.tensor.transpose(vt_ps[:, q, :],
                                    vT[:, jt * 128:(jt + 1) * 128], ident[:])
            nc.vector.tensor_copy(v_aug[:, 4 * tch:4 * tch + 4, 0:H], vt_ps[:])

        def emit_spair(b, g, on_pool=False):
            isl = slice(b * 512, (b + 1) * 512)
            st = psS.tile([128, 1024], F32, tag="st", name=f"stf{b}_{g}")
            for h in range(2):
                jt = 2 * g + h
                nc.tensor.matmul(st[:, 512 * h:512 * (h + 1)],
                                 k_sb[:, jt * 128:(jt + 1) * 128],
                                 q_sb[:, isl], start=True, stop=True)
            ef = epool.tile([128, 1024], BF16, tag="e", name=f"ef{b}_{g}")
            if on_pool:
                s2 = spool.tile([128, 1024], F32, tag="s2", name=f"s2_{b}_{g}")
                nc.vector.tensor_copy(s2[:], st[:])
                nc.gpsimd.tensor_tensor(ef[:], ce[:].broadcast_to((128, 1024)),
                                        s2[:], mybir.AluOpType.pow)
            else:
                nc.scalar.activation(ef[:], st[:], EXP)
            e_full[(b, g)] = ef

        def emit_sdiag(b, sl):
            ms = (0,) if sl == 0 else (1, 3, 2)
            st = psS.tile([128, 1024], F32, tag="st", name=f"std{b}_{sl}")
            used = 512 if sl == 0 else 768
            for m in ms:
                _, off, n = DIAG[m]
                jt = 4 * b + m
                nc.tensor.matmul(
                    st[:, off:off + n],
                    k_sb[:, jt * 128:(jt + 1) * 128],
                    q_sb[:, b * 512 + 128 * m:(b + 1) * 512],
                    start=True, stop=True)
            ed = epool.tile([128, 1024], BF16, tag="e", name=f"ed{b}_{sl}")
            nc.scalar.activation(ed[:, 0:used], st[:, 0:used], EXP)
            for m in ms:
                _, off, _ = DIAG[m]
                nc.vector.tensor_tensor(ed[:, off:off + 128],
                                        ed[:, off:off + 128], tri[:],
                                        mybir.AluOpType.mult)
            e_diag[(b, sl)] = ed

        def emit_pv(b, m_loc):
            if m_loc == 0:
                o_ps_all[b] = psO.tile([128, 4, H + 1], F32, tag="o",
                                       name=f"o_ps{b}")
            o_ps = o_ps_all[b]
            it = 4 * b + m_loc
            order = [4 * b + m for m in range(m_loc + 1)] + \
                    [jt for jt in range(4 * b)]
            for idx, jt in enumerate(order):
                if jt < 4 * b:
                    src = e_full[(b, jt // 2)]
                    col = 512 * (jt % 2) + 128 * m_loc
                else:
                    m = jt - 4 * b
                    sl, off, _ = DIAG[m]
                    src = e_diag[(b, sl)]
                    col = off + 128 * (m_loc - m)
                nc.tensor.matmul(o_ps[:, m_loc, :],
                                 src[:, col:col + 128],
                                 v_aug[:, jt, :],
                                 start=(idx == 0), stop=(idx == len(order) - 1))

        def emit_epilogue(b):
            o_ps = o_ps_all[b]
            rec = rpool.tile([128, 4], F32, tag="r")
            nc.vector.reciprocal(rec[:], o_ps[:, :, H:H + 1].rearrange(
                "p a o -> p (a o)"))
            rec_b = rec[:].unsqueeze(2).broadcast_to((128, 4, H))
            nc.vector.tensor_tensor(o_sb[:, 4 * b:4 * b + 4, :],
                                    o_ps[:, :, 0:H], rec_b,
                                    mybir.AluOpType.mult)
            isl = slice(b * 512, (b + 1) * 512)
            nc.sync.dma_start(
                out_d[isl, :].rearrange("(a p) h -> p a h", p=128),
                o_sb[:, 4 * b:4 * b + 4, :])

        # arrival-aware emission, order forced via tile_wait_until slots
        SCHED = [
            (4.7, lambda: emit_proj_kq(0)),
            (6.2, lambda: emit_kcopy(0, False)),
            (6.8, lambda: emit_sdiag(0, 0)),
            (7.0, lambda: emit_sdiag(0, 1)),
            (7.1, lambda: emit_proj_kq(2)),
            (7.9, lambda: emit_kcopy(2, False)),
            (7.8, lambda: emit_proj_v(0)),
            (9.4, lambda: emit_proj_kq(1)),
            (9.6, lambda: emit_kcopy(1, False)),
            (9.7, lambda: emit_sdiag(2, 0)),
            (9.9, lambda: emit_sdiag(2, 1)),
            (10.0, lambda: emit_spair(2, 0)),
            (10.2, lambda: emit_spair(2, 1)),
            (10.3, lambda: emit_pv(0, 0)),
            (10.35, lambda: emit_pv(0, 1)),
            (10.4, lambda: emit_pv(0, 2)),
            (10.45, lambda: emit_pv(0, 3)),
            (10.5, lambda: emit_proj_v(2)),
            (10.8, lambda: emit_sdiag(1, 0)),
            (11.0, lambda: emit_sdiag(1, 1)),
            (11.1, lambda: emit_spair(1, 0)),
            (11.3, lambda: emit_spair(1, 1)),
            (11.35, lambda: emit_epilogue(0)),
            (11.4, lambda: emit_proj_kq(3)),
            (11.55, lambda: emit_kcopy(3, False)),
            (11.6, lambda: emit_spair(2, 2)),
            (11.8, lambda: emit_spair(2, 3)),
            (11.9, lambda: emit_proj_v(1)),
            (12.3, lambda: emit_sdiag(3, 0)),
            (12.5, lambda: emit_sdiag(3, 1)),
            (12.6, lambda: emit_pv(1, 0)),
            (12.65, lambda: emit_pv(1, 1)),
            (12.7, lambda: emit_pv(1, 2)),
            (12.75, lambda: emit_pv(1, 3)),
            (12.8, lambda: emit_proj_v(3)),
            (12.9, lambda: emit_spair(3, 0, on_pool=True)),
            (13.1, lambda: emit_spair(3, 1)),
            (13.15, lambda: emit_epilogue(1)),
            (13.3, lambda: emit_pv(2, 0)),
            (13.35, lambda: emit_pv(2, 1)),
            (13.4, lambda: emit_pv(2, 2)),
            (13.45, lambda: emit_pv(2, 3)),
            (13.6, lambda: emit_spair(3, 2)),
            (13.8, lambda: emit_spair(3, 3)),
            (14.0, lambda: emit_spair(3, 4)),
            (14.2, lambda: emit_spair(3, 5)),
            (14.3, lambda: emit_epilogue(2)),
            (14.5, lambda: emit_pv(3, 0)),
            (14.55, lambda: emit_pv(3, 1)),
            (14.6, lambda: emit_pv(3, 2)),
            (14.65, lambda: emit_pv(3, 3)),
            (14.8, lambda: emit_epilogue(3)),
        ]
        for ts_us, fn in SCHED:
            with tc.tile_wait_until(ts_us * 1e-3):
                fn()

def build_nc(repeat=1):
    nc = bacc.Bacc("TRN2", target_bir_lowering=False, debug=False, num_devices=8)
    x_d = nc.dram_tensor("x", [T, D], BF16, kind="ExternalInput")
    w_d = nc.dram_tensor("w", [128, ND * 192 + 2], BF16, kind="ExternalInput")
    out_d = nc.dram_tensor("out", [T, H], BF16, kind="ExternalOutput")
    dram = (x_d, w_d, out_d)

    from contextlib import ExitStack
    with tile.TileContext(nc) as tc:
        with ExitStack() as ctx:
            build_body(nc, tc, ctx, dram, repeat=repeat)
    nc.compile()
    return nc


_NC_CACHE = {}


def _get_nc(repeat=1):
    if repeat not in _NC_CACHE:
        _NC_CACHE[repeat] = build_nc(repeat)
    return _NC_CACHE[repeat]


def make_in_maps(x, Wk, bk, Wq, bq, Wv, bv):
    scale = float(H) ** -0.5
    bf = ml_dtypes.bfloat16
    w = np.concatenate(
        [Wk.reshape(ND, 128, H), (Wq * scale).reshape(ND, 128, H),
         Wv.reshape(ND, 128, H)], axis=2)
    w = np.ascontiguousarray(w.transpose(1, 0, 2)).reshape(128, ND * 192)
    b = np.zeros((128, 2), dtype=np.float32)
    b[0:H, 0] = bq * scale
    b[0:H, 1] = bv
    w = np.concatenate([w, b], axis=1).astype(bf)
    xb = x.astype(bf)
    return [
        {"x": np.ascontiguousarray(xb[i]), "w": np.ascontiguousarray(w)}
        for i in range(B)
    ]


def kernel(x, Wk, bk, Wq, bq, Wv, bv, _repeat=1):
    x = np.asarray(x, dtype=np.float32)
    Wk = np.asarray(Wk, dtype=np.float32)
    bk = np.asarray(bk, dtype=np.float32)
    Wq = np.asarray(Wq, dtype=np.float32)
    bq = np.asarray(bq, dtype=np.float32)
    Wv = np.asarray(Wv, dtype=np.float32)
    bv = np.asarray(bv, dtype=np.float32)

    nc = _get_nc(_repeat)
    in_maps = make_in_maps(x, Wk, bk, Wq, bq, Wv, bv)
    res = run_bass_kernel_spmd(nc, in_maps, core_ids=list(range(B)))
    out = np.stack([np.asarray(res.results[i]["out"], dtype=np.float32)
                    for i in range(B)], axis=0)
    return out
